# revision 5
# baseline (speedup 1.0000x reference)
"""Multi-head attention (B=4, N=2048, D=1024, H=16) on 8 TRN2 NeuronCores.

Sharding: 8 cores = batch(4) x sequence-half(2). Each core computes the full
attention output for its 1024-token slice of one batch (all 16 heads), so the
final unshard is a pure gather. The only cross-core traffic is an AllGather of
K^T and V between the two cores of each batch pair.

Per-core pipeline (bf16 matmul operands, fp32 PSUM accumulation):
  1. Cast x / w_qkv / w_proj to bf16, stage to DRAM, and DMA-transpose back so
     contraction dims sit on SBUF partitions.
  2. QKV projection. Q^T and K^T are produced in [d_out, token] orientation
     (lhsT = w_qkv^T tile, rhs = x^T); V in natural [token, d] orientation
     (lhsT = x^T tile, rhs = w_qkv^T).
  3. AllGather K^T then V across the pair (k-token axis spans both halves).
  4. Attention per head-pair p: S^T = (QK^T)^T via row-paired matmuls
     (contraction = head_dim 64, two heads in array row halves), exp on
     ScalarE straight out of PSUM (logits are bounded, no max subtraction),
     then O^T and the softmax denominator via col-paired matmuls over the
     k axis. The all-ones denominator lhsT replicates each head's denominator
     across its 64 output partitions, so normalization is a single full-width
     reciprocal + multiply on VectorE.
  5. Output projection from the accumulated attout^T tiles, bias add, DMA out.
"""

import sys

for _p in ("/opt/trn_rl_repo",):
    if _p not in sys.path:
        sys.path.insert(0, _p)

import numpy as np

import concourse.bass as bass
import concourse.mybir as mybir
import concourse.tile as tile
from concourse import bacc
from concourse.bass_utils import run_bass_kernel_spmd

B, N, D, H, HD = 4, 2048, 1024, 16, 64
SCALE = HD ** -0.5
NL = N // 2  # tokens per core
NCORES = 8
RG = [[0, 1], [2, 3], [4, 5], [6, 7]]
F32 = mybir.dt.float32
BF16 = mybir.dt.bfloat16
EXP = mybir.ActivationFunctionType.Exp


def _emit(tc, aps):
    nc = tc.nc
    x_l, wqkv, wproj, bias, out = (
        aps["x_local"], aps["w_qkv"], aps["w_proj"], aps["b_proj"], aps["out"])
    wqkv_bf, wproj_bf, x_bf = aps["wqkv_bf"], aps["wproj_bf"], aps["x_bf"]
    cc_k, cc_v, k_g, v_g = aps["cc_k"], aps["cc_v"], aps["k_g"], aps["v_g"]

    persist1 = tc.alloc_tile_pool(name="persist1", bufs=1)

    # ---- Phase A: load fp32, cast to bf16, stage to DRAM ------------------
    with tc.tile_pool(name="prep", bufs=3) as prep:
        def cast_stage(src, dst, rows):
            for i in range(rows // 128):
                t = prep.tile([128, D], F32, tag="ld_f32")
                nc.sync.dma_start(out=t, in_=src[i * 128:(i + 1) * 128, :])
                tb = prep.tile([128, D], BF16, tag="cast_bf")
                nc.vector.tensor_copy(tb, t)
                nc.sync.dma_start(out=dst[i * 128:(i + 1) * 128, :], in_=tb)

        cast_stage(wqkv, wqkv_bf, 3 * D)
        cast_stage(wproj, wproj_bf, D)
        cast_stage(x_l, x_bf, NL)

    # bias broadcast-loaded across all 128 partitions (DMA re-reads DRAM row)
    bias_sb = persist1.tile([128, D], F32, tag="bias")
    bias_bcast = bass.AP(tensor=bias.tensor, offset=bias.offset,
                         ap=[[0, 128], *bias.ap])
    nc.sync.dma_start(out=bias_sb, in_=bias_bcast)

    ones_sb = persist1.tile([128, 64], BF16, tag="ones")
    nc.vector.memset(ones_sb, 1.0)

    # persistent attention operands
    qT = [persist1.tile([128, NL], BF16, tag=f"qT{p}", name=f"qT{p}") for p in range(8)]
    kT = [persist1.tile([128, N], BF16, tag=f"kT{p}", name=f"kT{p}") for p in range(8)]
    vv = [persist1.tile([128, D], BF16, tag=f"v{kt}", name=f"v{kt}") for kt in range(16)]

    # ---- Phase B/C: transposed loads + QKV projections --------------------
    with tc.tile_pool(name="qkv", bufs=1) as qkvp, \
         tc.tile_pool(name="qkv_ps", bufs=4, space="PSUM") as qkvps, \
         tc.tile_pool(name="qkv_sb", bufs=3) as qkvsb:
        xT = [qkvp.tile([128, NL], BF16, tag=f"xT{k}", name=f"xT{k}") for k in range(8)]
        for k in range(8):
            nc.sync.dma_start_transpose(out=xT[k], in_=x_bf[:, k * 128:(k + 1) * 128])
        wT = [qkvp.tile([128, 3 * D], BF16, tag=f"wT{k}", name=f"wT{k}") for k in range(8)]
        for k in range(8):
            nc.sync.dma_start_transpose(out=wT[k], in_=wqkv_bf[:, k * 128:(k + 1) * 128])

        def proj_dT(m, dst_sb):
            # dst_sb[:, :] = (w_qkv rows m*128..)^T @ x^T  -> [d_out 128, NL]
            for qc in range(2):
                ps = qkvps.tile([128, 512], F32, tag="qkv_ps")
                for k in range(8):
                    nc.tensor.matmul(
                        out=ps,
                        lhsT=wT[k][:, m * 128:(m + 1) * 128],
                        rhs=xT[k][:, qc * 512:(qc + 1) * 512],
                        start=(k == 0), stop=(k == 7))
                nc.vector.tensor_copy(dst_sb[:, qc * 512:(qc + 1) * 512], ps)

        # K rows (d_out 1024:2048) first so the K AllGather launches early
        for m in range(8, 16):
            ksb = qkvsb.tile([128, NL], BF16, tag="k_loc")
            proj_dT(m, ksb)
            nc.sync.dma_start(out=cc_k[(m - 8) * 128:(m - 7) * 128, :], in_=ksb)
        nc.gpsimd.collective_compute(
            "AllGather", mybir.AluOpType.bypass, replica_groups=RG,
            ins=[cc_k], outs=[k_g])

        for m in range(8):
            proj_dT(m, qT[m])

        # V in natural [token, d] orientation
        for t in range(8):
            vsb = qkvsb.tile([128, D], BF16, tag="v_loc")
            for vc in range(2):
                ps = qkvps.tile([128, 512], F32, tag="qkv_ps")
                for k in range(8):
                    nc.tensor.matmul(
                        out=ps,
                        lhsT=xT[k][:, t * 128:(t + 1) * 128],
                        rhs=wT[k][:, 2 * D + vc * 512:2 * D + (vc + 1) * 512],
                        start=(k == 0), stop=(k == 7))
                nc.vector.tensor_copy(vsb[:, vc * 512:(vc + 1) * 512], ps)
            nc.sync.dma_start(out=cc_v[t * 128:(t + 1) * 128, :], in_=vsb)
        nc.gpsimd.collective_compute(
            "AllGather", mybir.AluOpType.bypass, replica_groups=RG,
            ins=[cc_v], outs=[v_g])

        # gathered loads: rank 0 = tokens 0:NL, rank 1 = NL:N (both cores agree)
        for p in range(8):
            nc.sync.dma_start(out=kT[p][:, 0:NL], in_=k_g[0, p * 128:(p + 1) * 128, :])
            nc.sync.dma_start(out=kT[p][:, NL:N], in_=k_g[1, p * 128:(p + 1) * 128, :])
        for kt in range(16):
            nc.sync.dma_start(
                out=vv[kt], in_=v_g[kt // 8, (kt % 8) * 128:(kt % 8 + 1) * 128, :])

    # ---- Phase D: attention ----------------------------------------------
    persist2 = tc.alloc_tile_pool(name="persist2", bufs=1)
    _persist2_open = True
    attoutT = [persist2.tile([128, NL], BF16, tag=f"ao{p}", name=f"ao{p}") for p in range(8)]
    wpT = [persist2.tile([128, D], BF16, tag=f"wpT{k}", name=f"wpT{k}") for k in range(8)]
    for k in range(8):
        nc.sync.dma_start_transpose(out=wpT[k], in_=wproj_bf[:, k * 128:(k + 1) * 128])

    with tc.tile_pool(name="s_ps", bufs=2, space="PSUM") as spool, \
         tc.tile_pool(name="o_ps", bufs=2, space="PSUM") as opool, \
         tc.tile_pool(name="pT", bufs=18) as ppool, \
         tc.tile_pool(name="rc", bufs=2) as rpool:
        for p in range(8):
            for qc in range(2):
                # S + exp for all 16 k-tiles (PE stays in row-tiled mode; the
                # O/den col-mode matmuls are batched after to avoid per-ktile
                # PE tiling-mode drains)
                pts = []
                for kt in range(16):
                    s = spool.tile([128, 2, 512], F32, tag="s_ps")
                    for h in range(2):
                        # S^T[k_tok, q] for head 2p+h; contraction over HD=64
                        nc.tensor.matmul(
                            out=s[:, h, :],
                            lhsT=kT[p][h * 64:(h + 1) * 64, kt * 128:(kt + 1) * 128],
                            rhs=qT[p][h * 64:(h + 1) * 64, qc * 512:(qc + 1) * 512],
                            start=True, stop=True,
                            tile_position=(h * 64, 0))
                    pt = ppool.tile([128, 2, 512], BF16, tag="pT")
                    nc.scalar.activation(pt, s, EXP, scale=SCALE)
                    pts.append(pt)
                o = opool.tile([128, 512], F32, tag="o_ps")
                dn = opool.tile([128, 512], F32, tag="den_ps")
                for kt in range(16):
                    for h in range(2):
                        nc.tensor.matmul(
                            out=o[h * 64:(h + 1) * 64, :],
                            lhsT=vv[kt][:, (2 * p + h) * 64:(2 * p + h + 1) * 64],
                            rhs=pts[kt][:, h, :],
                            start=(kt == 0), stop=(kt == 15),
                            tile_position=(0, h * 64))
                for kt in range(16):
                    for h in range(2):
                        nc.tensor.matmul(
                            out=dn[h * 64:(h + 1) * 64, :],
                            lhsT=ones_sb,
                            rhs=pts[kt][:, h, :],
                            start=(kt == 0), stop=(kt == 15),
                            tile_position=(0, h * 64))
                rc = rpool.tile([128, 512], F32, tag="rc")
                nc.vector.reciprocal(rc, dn)
                nc.vector.tensor_mul(attoutT[p][:, qc * 512:(qc + 1) * 512], o, rc)

    # ---- Phase E: output projection + bias --------------------------------
    with tc.tile_pool(name="proj_ps", bufs=4, space="PSUM") as projps, \
         tc.tile_pool(name="y_sb", bufs=3) as ypool:
        for tt in range(8):
            for ec in range(2):
                ps = projps.tile([128, 512], F32, tag="proj_ps")
                for p in range(8):
                    nc.tensor.matmul(
                        out=ps,
                        lhsT=attoutT[p][:, tt * 128:(tt + 1) * 128],
                        rhs=wpT[p][:, ec * 512:(ec + 1) * 512],
                        start=(p == 0), stop=(p == 7))
                yt = ypool.tile([128, 512], F32, tag="y_sb")
                nc.vector.tensor_add(yt, ps, bias_sb[:, ec * 512:(ec + 1) * 512])
                nc.sync.dma_start(
                    out=out[tt * 128:(tt + 1) * 128, ec * 512:(ec + 1) * 512],
                    in_=yt)
    persist2.release()
    persist1.release()


def _build():
    nc = bacc.Bacc("TRN2", target_bir_lowering=False, debug=False,
                   num_devices=NCORES)
    aps = {
        "x_local": nc.dram_tensor("x_local", [NL, D], F32, kind="ExternalInput").ap(),
        "w_qkv": nc.dram_tensor("w_qkv", [3 * D, D], F32, kind="ExternalInput").ap(),
        "w_proj": nc.dram_tensor("w_proj", [D, D], F32, kind="ExternalInput").ap(),
        "b_proj": nc.dram_tensor("b_proj", [D], F32, kind="ExternalInput").ap(),
        "out": nc.dram_tensor("out", [NL, D], F32, kind="ExternalOutput").ap(),
        "wqkv_bf": nc.dram_tensor("wqkv_bf", [3 * D, D], BF16).ap(),
        "wproj_bf": nc.dram_tensor("wproj_bf", [D, D], BF16).ap(),
        "x_bf": nc.dram_tensor("x_bf", [NL, D], BF16).ap(),
        "cc_k": nc.dram_tensor("cc_k", [D, NL], BF16).ap(),
        "cc_v": nc.dram_tensor("cc_v", [NL, D], BF16).ap(),
        "k_g": nc.dram_tensor("k_g", [2, D, NL], BF16).ap(),
        "v_g": nc.dram_tensor("v_g", [2, NL, D], BF16).ap(),
    }
    with tile.TileContext(nc) as tc:
        _emit(tc, aps)
    nc.compile()
    return nc


_NC = None


def _get_nc():
    global _NC
    if _NC is None:
        _NC = _build()
    return _NC


def run(x, w_qkv, w_proj, b_proj, **spmd_kwargs):
    nc = _get_nc()
    x = np.ascontiguousarray(np.asarray(x, dtype=np.float32))
    w_qkv = np.ascontiguousarray(np.asarray(w_qkv, dtype=np.float32))
    w_proj = np.ascontiguousarray(np.asarray(w_proj, dtype=np.float32))
    b_proj = np.ascontiguousarray(np.asarray(b_proj, dtype=np.float32))
    in_maps = []
    for c in range(NCORES):
        b, half = divmod(c, 2)
        in_maps.append({
            "x_local": np.ascontiguousarray(x[b, half * NL:(half + 1) * NL, :]),
            "w_qkv": w_qkv,
            "w_proj": w_proj,
            "b_proj": b_proj,
        })
    res = run_bass_kernel_spmd(nc, in_maps, list(range(NCORES)), **spmd_kwargs)
    y = np.empty((B, N, D), dtype=np.float32)
    for c in range(NCORES):
        b, half = divmod(c, 2)
        y[b, half * NL:(half + 1) * NL, :] = res.results[c]["out"]
    return y, res


def kernel(x, w_qkv, w_proj, b_proj):
    y, _ = run(x, w_qkv, w_proj, b_proj)
    return y
